# revision 21
# baseline (speedup 1.0000x reference)
"""Bass/Trainium2 kernel for DoubleGRUModel (ragged double GRU + linear head).

Contract: kernel(**inputs) takes FULL unsharded inputs (as numpy-compatible
arrays, keyed as in setup_inputs()) and returns the FULL [64, 1] float32
output. Internally shards the batch across 8 NeuronCores (8 items/core),
runs an SPMD Bass kernel, and gathers.

Per-core pipeline (all phases on-device, no cross-core traffic):
  A: GI1[t,i,:] = x @ Wi1.T + (bi1+bh1 for r,z; bi1 for n)   (batched matmul)
  B: layer-1 time recurrence; stores frozen h1.T per step for phase C
  C: GI2 = h1n @ Wi2.T + biases                               (batched matmul)
  D: layer-2 time recurrence (freeze folded into z-gate)
  E: head: (h2 @ W1.T + b1) @ W2.T + b2
"""

import numpy as np

B, T, I, H = 64, 2048, 256, 512
NCORES = 8
IB = B // NCORES          # items per core = 8
BLK = 8                   # time steps per For_i body
NB = T // BLK             # 256 blocks

_compiled = None


def _build(t_steps=T, dbg=False):
    import concourse.bacc as bacc
    import concourse.bass as bass
    import concourse.mybir as mybir
    import concourse.tile as tile
    from contextlib import ExitStack

    fp16 = mybir.dt.float16
    fp32 = mybir.dt.float32
    AF = mybir.ActivationFunctionType
    OP = mybir.AluOpType
    ds = bass.ds
    ts = bass.ts

    nb = t_steps // BLK
    rows = IB * t_steps          # (i, t)-major flattened rows
    n_mtiles = rows // 128

    nc = bacc.Bacc("TRN2", target_bir_lowering=False, debug=False,
                   num_devices=NCORES)

    def din(name, shape, dt=fp16):
        return nc.dram_tensor(name, list(shape), dt, kind="ExternalInput").ap()

    # ---- external inputs (host-prepped layouts) ----
    xT = din("xT", [2, 128, rows])                   # x.T k-chunks
    wi1 = din("wi1", [2, 128, 1536])                 # Wi1.T k-chunks (gate cols r,z,n)
    bias1 = din("bias1", [1, 1536])                  # bi1 + (bh1 for r,z)
    bh1n = din("bh1n", [1, 512])                     # bh1 n-slice
    wh1 = din("wh1", [4, 128, 1536])                 # Wh1.T k-chunks
    wi2 = din("wi2", [4, 128, 1536])
    bias2 = din("bias2", [1, 1536])
    bh2n = din("bh2n", [1, 512])
    wh2 = din("wh2", [4, 128, 1536])
    w1t = din("w1t", [4, 128, 128])                  # W1.T
    b1r = din("b1r", [1, 128])
    w2rep = din("w2rep", [IB, 128], fp32)            # W2 row replicated
    b2rep = din("b2rep", [IB, 1], fp32)
    h10 = din("h10", [IB, 512])                      # initial states (bcast)
    h20 = din("h20", [IB, 512])
    h10t = din("h10t", [128, 4, IB])                 # same, transposed
    h20t = din("h20t", [128, 4, IB])
    ident = din("ident", [IB, IB])                   # identity matrix
    ident3 = din("ident3", [72, IB])                 # identity at partitions {0,32,64}
    ones1 = din("ones1", [1, 128])                   # ones rows
    mask1 = din("mask1", [nb, IB, BLK], fp32)              # alive(t)=1 else 0
    mask2 = din("mask2", [nb, IB, 2, BLK], fp32)           # [alive, dead]
    y = nc.dram_tensor("y", [IB, 1], fp32, kind="ExternalOutput").ap()
    if dbg:
        rows_ = IB * t_steps
        y_gi1 = nc.dram_tensor("y_gi1", [3, rows_, 512], fp16,
                               kind="ExternalOutput").ap()
        y_h1 = nc.dram_tensor("y_h1", [IB, 512], fp16,
                              kind="ExternalOutput").ap()
        y_h1t = nc.dram_tensor("y_h1t", [4, 128, rows_], fp16,
                               kind="ExternalOutput").ap()
        y_h2 = nc.dram_tensor("y_h2", [IB, 512], fp16,
                              kind="ExternalOutput").ap()

    # ---- internal DRAM ----
    gi1 = nc.dram_tensor("gi1", [3, rows, 512], fp16).ap()
    gi2 = nc.dram_tensor("gi2", [3, rows, 512], fp16).ap()
    h1t_d = nc.dram_tensor("h1t_d", [4, 128, rows], fp16).ap()

    EPE = mybir.EngineType

    with tile.TileContext(nc) as tc, ExitStack() as top:
        singles = top.enter_context(tc.tile_pool(name="singles", bufs=1))

        # ---- persistent SBUF: weights, constants, states ----
        def load_single(src, shape, name, dt=fp16):
            t_ = singles.tile(list(shape), dt, tag=name, name=name)
            if len(shape) == 3:
                # dram [c, 128, f] -> sbuf [128, c, f]
                nc.sync.dma_start(out=t_, in_=src.rearrange("c p f -> p c f"))
            else:
                nc.sync.dma_start(out=t_, in_=src)
            return t_

        wh1_sb = load_single(wh1, [128, 4, 1536], "wh1")
        wh2_sb = load_single(wh2, [128, 4, 1536], "wh2")
        wi1_sb = load_single(wi1, [128, 2, 1536], "wi1")
        wi2_sb = load_single(wi2, [128, 4, 1536], "wi2")
        w1t_sb = load_single(w1t, [128, 4, 128], "w1t")
        bias1_sb = load_single(bias1, [1, 1536], "bias1")
        bias2_sb = load_single(bias2, [1, 1536], "bias2")
        bh1n_sb = load_single(bh1n, [1, 512], "bh1n")
        bh2n_sb = load_single(bh2n, [1, 512], "bh2n")
        b1r_sb = load_single(b1r, [1, 128], "b1r")
        w2_sb = load_single(w2rep, [IB, 128], "w2rep", fp32)
        b2_sb = load_single(b2rep, [IB, 1], "b2rep", fp32)
        ident_sb = load_single(ident, [IB, IB], "ident")
        ident3_sb = load_single(ident3, [72, IB], "ident3")
        ones1_sb = load_single(ones1, [1, 128], "ones1")

        h_sb = [singles.tile([IB, 512], fp16, tag=f"h{k}", name=f"h{k}")
                for k in range(2)]
        ht_sb = [singles.tile([128, 4, IB], fp16, tag=f"ht{k}", name=f"ht{k}")
                 for k in range(2)]

        # ===================== phase A / C: batched GI =====================
        def gi_phase(lhs_dram, k_chunks, w_sb, bias_sb, gi_out, unroll):
            with ExitStack() as ctx:
                lp = ctx.enter_context(tc.tile_pool(name="gilhs", bufs=3))
                pp = ctx.enter_context(tc.tile_pool(name="gips", bufs=2, space="PSUM"))
                op_ = ctx.enter_context(tc.tile_pool(name="giout", bufs=3))
                with tc.For_i(0, n_mtiles // unroll, 1,
                              hint_engines=(EPE.PE, EPE.SP)) as mb:
                    for u in range(unroll):
                        m = mb * unroll + u
                        lhs = []
                        for k in range(k_chunks):
                            lt = lp.tile([128, 128], fp16, tag=f"lhs{k}",
                                         name=f"lhs{k}")
                            nc.sync.dma_start(out=lt, in_=lhs_dram[k, :, ts(m, 128)])
                            lhs.append(lt)
                        ps = pp.tile([128, 3, 512], fp32, tag="ps")
                        for g in range(3):
                            for k in range(k_chunks):
                                nc.tensor.matmul(ps[:, g, :], lhs[k],
                                                 w_sb[:, k, g * 512:(g + 1) * 512],
                                                 start=(k == 0), stop=False)
                            nc.tensor.matmul(ps[:, g, :], ones1_sb[0:1, :],
                                             bias_sb[0:1, g * 512:(g + 1) * 512],
                                             start=False, stop=True)
                        ot = op_.tile([128, 3, 512], fp16, tag="ot")
                        nc.vector.tensor_copy(ot, ps)
                        for g in range(3):
                            nc.gpsimd.dma_start(out=gi_out[g, ts(m, 128), :],
                                                in_=ot[:, g, :])

        gi_phase(xT, 2, wi1_sb, bias1_sb, gi1, unroll=min(8, n_mtiles))

        # ===================== phase B / D: recurrence =====================
        def recurrence(layer, gi_d, wh_sb, bhn_sb, m1_d, m2_d, hp, htp, h1t_out):
            gi_v = gi_d.rearrange("g (i nb j) f -> g i nb j f", i=IB, j=BLK)
            if h1t_out is not None:
                h1t_v = h1t_out.rearrange("c p (i t) -> c p i t", i=IB)
            with ExitStack() as ctx:
                gp = ctx.enter_context(tc.tile_pool(name="giblk", bufs=2))
                mp = ctx.enter_context(tc.tile_pool(name="mblk", bufs=2))
                pp = ctx.enter_context(tc.tile_pool(name="rpsum", bufs=2, space="PSUM"))
                tp = ctx.enter_context(tc.tile_pool(name="rtmp", bufs=2))
                pt = ctx.enter_context(tc.tile_pool(name="rpsT", bufs=2, space="PSUM"))
                with tc.For_i(0, nb, 1,
                              hint_engines=(EPE.PE, EPE.DVE, EPE.Activation,
                                            EPE.Pool, EPE.SP)) as ib:
                    gi_blk = gp.tile([72, BLK, 512], fp16, tag="gi")
                    for g in range(3):
                        nc.sync.dma_start(
                            out=gi_blk[32 * g:32 * g + IB, :, :],
                            in_=gi_v[g, :, ds(ib, 1), :, :])
                    if layer == 1:
                        mk = mp.tile([IB, BLK], fp32, tag="mk")
                        nc.sync.dma_start(out=mk, in_=m1_d[ds(ib, 1), :, :])
                    else:
                        mk = mp.tile([IB, 2, BLK], fp32, tag="mk")
                        nc.sync.dma_start(out=mk, in_=m2_d[ds(ib, 1), :, :, :])
                    for j in range(BLK):
                        h_cur, h_nxt = h_sb[(hp + j) % 2], h_sb[(hp + j + 1) % 2]
                        ht_cur, ht_nxt = ht_sb[(htp + j) % 2], ht_sb[(htp + j + 1) % 2]
                        # gh matmul, gates col-tiled to psum partitions {0,32,64};
                        # gi for r,z folded into PSUM via identity matmuls
                        ps = pp.tile([96, 512], fp32, tag="ps")
                        for g in range(3):
                            for k in range(4):
                                nc.tensor.matmul(
                                    ps[32 * g:32 * g + IB, :], ht_cur[:, k, :],
                                    wh_sb[:, k, g * 512:(g + 1) * 512],
                                    start=(k == 0), stop=False)
                            if g != 2:
                                nc.tensor.matmul(
                                    ps[32 * g:32 * g + IB, :],
                                    ident3_sb[32 * g:32 * g + IB, :],
                                    gi_blk[32 * g:32 * g + IB, j, :],
                                    start=False, stop=True)
                        nc.tensor.matmul(ps[64:64 + IB, :], ones1_sb[0:1, 0:IB],
                                         bhn_sb[0:1, :], start=False, stop=True)
                        # gates: sigmoids read summed PSUM directly
                        rbuf = tp.tile([72, 512], fp16, tag="rb")
                        zbuf = tp.tile([IB, 512], fp16, tag="zb")
                        nc.scalar.activation(rbuf[64:64 + IB, :], ps[0:IB, :],
                                             AF.Sigmoid)
                        nc.scalar.activation(zbuf, ps[32:32 + IB, :], AF.Sigmoid)
                        hnb = tp.tile([72, 512], fp16, tag="hnb")
                        nc.scalar.copy(hnb[64:64 + IB, :], ps[64:64 + IB, :])
                        t1 = tp.tile([72, 512], fp16, tag="t1")
                        nc.vector.tensor_mul(t1[64:64 + IB, :], rbuf[64:64 + IB, :],
                                             hnb[64:64 + IB, :])
                        an = tp.tile([72, 512], fp16, tag="an")
                        nc.gpsimd.tensor_tensor(an[64:64 + IB, :], t1[64:64 + IB, :],
                                                gi_blk[64:64 + IB, j, :], OP.add)
                        nbf = tp.tile([IB, 512], fp16, tag="nb")
                        nc.scalar.activation(nbf, an[64:64 + IB, :], AF.Tanh)
                        if layer == 2:
                            # fold freeze into z: z' = z*alive + dead
                            zm = tp.tile([IB, 512], fp16, tag="zm")
                            nc.vector.tensor_scalar(zm, zbuf, mk[:, 0, j:j + 1],
                                                    mk[:, 1, j:j + 1],
                                                    op0=OP.mult, op1=OP.add)
                            zbuf = zm
                        d = tp.tile([IB, 512], fp16, tag="d")
                        nc.gpsimd.tensor_tensor(d, h_cur, nbf, OP.subtract)
                        zd = tp.tile([IB, 512], fp16, tag="zd")
                        nc.vector.tensor_mul(zd, zbuf, d)
                        if layer == 1:
                            h1n = tp.tile([IB, 512], fp16, tag="h1n")
                            nc.vector.tensor_add(h1n, nbf, zd)
                            d2 = tp.tile([IB, 512], fp16, tag="d2")
                            nc.gpsimd.tensor_tensor(d2, h1n, h_cur, OP.subtract)
                            nc.vector.scalar_tensor_tensor(
                                h_nxt, d2, mk[:, j:j + 1], h_cur,
                                op0=OP.mult, op1=OP.add)
                        else:
                            nc.vector.tensor_add(h_nxt, nbf, zd)
                        # transpose bridge: h_nxt [IB,512] -> ht_nxt [128,4,IB]
                        pst = pt.tile([128, 4, IB], fp16, tag="pst")
                        for c in range(4):
                            nc.tensor.transpose(pst[:, c, :],
                                                h_nxt[:, 128 * c:128 * (c + 1)],
                                                ident_sb)
                        nc.vector.tensor_copy(ht_nxt, pst)
                        if h1t_out is not None:
                            for c in range(4):
                                nc.gpsimd.dma_start(
                                    out=h1t_v[c, :, :, ds(ib * BLK + j, 1)],
                                    in_=ht_nxt[:, c, :])

        # init states
        nc.sync.dma_start(out=h_sb[0], in_=h10)
        nc.sync.dma_start(out=ht_sb[0], in_=h10t)
        recurrence(1, gi1, wh1_sb, bh1n_sb, mask1, None, 0, 0, h1t_d)

        if dbg:
            nc.sync.dma_start(out=y_gi1, in_=gi1)
            nc.sync.dma_start(out=y_h1, in_=h_sb[t_steps % 2])
            nc.sync.dma_start(out=y_h1t, in_=h1t_d)

        gi_phase(h1t_d, 4, wi2_sb, bias2_sb, gi2, unroll=min(4, n_mtiles))

        nc.sync.dma_start(out=h_sb[0], in_=h20)
        nc.sync.dma_start(out=ht_sb[0], in_=h20t)
        recurrence(2, gi2, wh2_sb, bh2n_sb, None, mask2, 0, 0, None)
        if dbg:
            nc.sync.dma_start(out=y_h2, in_=h_sb[t_steps % 2])

        # ===================== phase E: head =====================
        with ExitStack() as ctx:
            ep = ctx.enter_context(tc.tile_pool(name="ep", bufs=1))
            epp = ctx.enter_context(tc.tile_pool(name="epp", bufs=1, space="PSUM"))
            ps1 = epp.tile([IB, 128], fp32)
            hf = h_sb[t_steps % 2]
            htf = ht_sb[t_steps % 2]
            for k in range(4):
                nc.tensor.matmul(ps1, htf[:, k, :], w1t_sb[:, k, :],
                                 start=(k == 0), stop=False)
            nc.tensor.matmul(ps1, ones1_sb[0:1, 0:IB], b1r_sb[0:1, :],
                             start=False, stop=True)
            o1 = ep.tile([IB, 128], fp32)
            nc.vector.tensor_copy(o1, ps1)
            t2 = ep.tile([IB, 128], fp32)
            nc.vector.tensor_mul(t2, o1, w2_sb)
            red = ep.tile([IB, 1], fp32)
            nc.vector.tensor_reduce(red, t2, axis=mybir.AxisListType.X, op=OP.add)
            out_t = ep.tile([IB, 1], fp32)
            nc.vector.tensor_scalar(out_t, red, b2_sb[:, 0:1], None, op0=OP.add)
            nc.sync.dma_start(out=y, in_=out_t)

        _ = hf  # silence linter; hf used above

    nc.compile()
    return nc


def _prep_core(c, x, lengths, state0, state1, Wi1, Wh1, bi1, bh1,
               Wi2, Wh2, bi2, bh2, W1, b1, W2, b2, t_steps=T):
    f16 = np.float16
    nb = t_steps // BLK
    xs = np.asarray(x)[c * IB:(c + 1) * IB, :t_steps]        # [IB, T, I]
    ls = np.asarray(lengths)[c * IB:(c + 1) * IB]            # [IB]
    rows = IB * t_steps
    xT = np.ascontiguousarray(
        xs.reshape(rows, I).T.astype(f16)).reshape(2, 128, rows)

    def kstack(WT, kc):       # [K,3H] -> [kc,128,1536]
        return np.ascontiguousarray(WT.astype(f16)).reshape(kc, 128, 1536)

    Wi1T = np.asarray(Wi1).T.astype(np.float32)              # [256, 1536]
    Wh1T = np.asarray(Wh1).T.astype(np.float32)              # [512, 1536]
    Wi2T = np.asarray(Wi2).T.astype(np.float32)
    Wh2T = np.asarray(Wh2).T.astype(np.float32)
    bi1_, bh1_ = np.asarray(bi1, np.float32), np.asarray(bh1, np.float32)
    bi2_, bh2_ = np.asarray(bi2, np.float32), np.asarray(bh2, np.float32)
    bias1 = bi1_.copy(); bias1[:1024] += bh1_[:1024]
    bias2 = bi2_.copy(); bias2[:1024] += bh2_[:1024]

    tgrid = np.arange(t_steps)
    alive = (tgrid[None, :] < ls[:, None]).astype(np.float32)       # [IB, T]
    m1 = np.ascontiguousarray(alive.reshape(IB, nb, BLK).transpose(1, 0, 2))
    m2 = np.stack([alive, 1.0 - alive], axis=1).astype(np.float32)  # [IB,2,T]
    m2 = np.ascontiguousarray(
        m2.reshape(IB, 2, nb, BLK).transpose(2, 0, 1, 3))

    s0 = np.broadcast_to(np.asarray(state0, np.float32), (IB, H)).astype(f16)
    s1 = np.broadcast_to(np.asarray(state1, np.float32), (IB, H)).astype(f16)
    s0t = np.ascontiguousarray(s0.T.reshape(4, 128, IB).transpose(1, 0, 2))
    s1t = np.ascontiguousarray(s1.T.reshape(4, 128, IB).transpose(1, 0, 2))

    W1T = np.asarray(W1).T.astype(f16)                       # [512, 128]
    return {
        "xT": xT,
        "wi1": kstack(Wi1T, 2), "bias1": bias1.astype(f16)[None, :],
        "bh1n": bh1_[1024:].astype(f16)[None, :],
        "wh1": kstack(Wh1T, 4),
        "wi2": kstack(Wi2T, 4), "bias2": bias2.astype(f16)[None, :],
        "bh2n": bh2_[1024:].astype(f16)[None, :],
        "wh2": kstack(Wh2T, 4),
        "w1t": np.ascontiguousarray(W1T).reshape(4, 128, 128),
        "b1r": np.asarray(b1, f16)[None, :],
        "w2rep": np.broadcast_to(np.asarray(W2, np.float32)[0], (IB, 128)).copy(),
        "b2rep": np.full((IB, 1), float(np.asarray(b2).reshape(-1)[0]), np.float32),
        "h10": s0, "h20": s1, "h10t": s0t, "h20t": s1t,
        "ident": np.eye(IB, dtype=f16),
        "ident3": np.concatenate([
            np.concatenate([np.eye(IB), np.zeros((24, IB))], axis=0)
            for _ in range(3)], axis=0)[:72].astype(f16),
        "ones1": np.ones((1, 128), f16),
        "mask1": m1, "mask2": m2,
    }


def kernel(**inputs):
    global _compiled
    from concourse.bass_utils import run_bass_kernel_spmd
    if _compiled is None:
        _compiled = _build()
    in_maps = [_prep_core(c, **inputs) for c in range(NCORES)]
    res = run_bass_kernel_spmd(_compiled, in_maps, core_ids=list(range(NCORES)))
    out = np.concatenate([res.results[c]["y"] for c in range(NCORES)], axis=0)
    return out.astype(np.float32)


# revision 27
# speedup vs baseline: 1.0160x; 1.0160x over previous
"""Bass/Trainium2 kernel for DoubleGRUModel (ragged double GRU + linear head).

Contract: kernel(**inputs) takes FULL unsharded inputs (as numpy-compatible
arrays, keyed as in setup_inputs()) and returns the FULL [64, 1] float32
output. Internally shards the batch across 8 NeuronCores (8 items/core),
runs an SPMD Bass kernel, and gathers.

Per-core pipeline (all phases on-device, no cross-core traffic):
  A: GI1[t,i,:] = x @ Wi1.T + (bi1+bh1 for r,z; bi1 for n)   (batched matmul)
  B: layer-1 time recurrence; stores frozen h1.T per step for phase C
  C: GI2 = h1n @ Wi2.T + biases                               (batched matmul)
  D: layer-2 time recurrence (freeze folded into z-gate)
  E: head: (h2 @ W1.T + b1) @ W2.T + b2
"""

import numpy as np

B, T, I, H = 64, 2048, 256, 512
NCORES = 8
IB = B // NCORES          # items per core = 8
BLK = 8                   # time steps per For_i body
NB = T // BLK             # 256 blocks

_compiled = None


def _build(t_steps=T, dbg=False):
    import concourse.bacc as bacc
    import concourse.bass as bass
    import concourse.mybir as mybir
    import concourse.tile as tile
    from contextlib import ExitStack

    fp16 = mybir.dt.float16
    fp32 = mybir.dt.float32
    AF = mybir.ActivationFunctionType
    OP = mybir.AluOpType
    ds = bass.ds
    ts = bass.ts

    nb = t_steps // BLK
    rows = IB * t_steps          # (i, t)-major flattened rows
    n_mtiles = rows // 128

    nc = bacc.Bacc("TRN2", target_bir_lowering=False, debug=False,
                   num_devices=NCORES)

    def din(name, shape, dt=fp16):
        return nc.dram_tensor(name, list(shape), dt, kind="ExternalInput").ap()

    # ---- external inputs (host-prepped layouts) ----
    xT = din("xT", [2, 128, rows])                   # x.T k-chunks
    wi1 = din("wi1", [2, 128, 1536])                 # Wi1.T k-chunks (gate cols r,z,n)
    bias1 = din("bias1", [1, 1536])                  # bi1 + (bh1 for r,z)
    bh1n = din("bh1n", [1, 512])                     # bh1 n-slice
    wh1 = din("wh1", [4, 128, 1536])                 # Wh1.T k-chunks
    wi2 = din("wi2", [4, 128, 1536])
    bias2 = din("bias2", [1, 1536])
    bh2n = din("bh2n", [1, 512])
    wh2 = din("wh2", [4, 128, 1536])
    w1t = din("w1t", [4, 128, 128])                  # W1.T
    b1r = din("b1r", [1, 128])
    w2rep = din("w2rep", [IB, 128], fp32)            # W2 row replicated
    b2rep = din("b2rep", [IB, 1], fp32)
    h10 = din("h10", [IB, 512])                      # initial states (bcast)
    h20 = din("h20", [IB, 512])
    h10t = din("h10t", [128, 4, IB])                 # same, transposed
    h20t = din("h20t", [128, 4, IB])
    ident = din("ident", [IB, IB])                   # identity matrix
    ident3 = din("ident3", [72, IB])                 # identity at partitions {0,32,64}
    ones1 = din("ones1", [1, 128])                   # ones rows
    mask1 = din("mask1", [nb, IB, BLK], fp32)              # alive(t)=1 else 0
    mask2 = din("mask2", [nb, IB, 2, BLK], fp32)           # [alive, dead]
    y = nc.dram_tensor("y", [IB, 1], fp32, kind="ExternalOutput").ap()
    if dbg:
        rows_ = IB * t_steps
        y_gi1 = nc.dram_tensor("y_gi1", [3, rows_, 512], fp16,
                               kind="ExternalOutput").ap()
        y_h1 = nc.dram_tensor("y_h1", [IB, 512], fp16,
                              kind="ExternalOutput").ap()
        y_h1t = nc.dram_tensor("y_h1t", [4, 128, rows_], fp16,
                               kind="ExternalOutput").ap()
        y_h2 = nc.dram_tensor("y_h2", [IB, 512], fp16,
                              kind="ExternalOutput").ap()

    # ---- internal DRAM ----
    gi1 = nc.dram_tensor("gi1", [3, rows, 512], fp16).ap()
    gi2 = nc.dram_tensor("gi2", [3, rows, 512], fp16).ap()
    h1t_d = nc.dram_tensor("h1t_d", [4, 128, rows], fp16).ap()

    EPE = mybir.EngineType

    with tile.TileContext(nc) as tc, ExitStack() as top:
        singles = top.enter_context(tc.tile_pool(name="singles", bufs=1))

        # ---- persistent SBUF: weights, constants, states ----
        def load_single(src, shape, name, dt=fp16):
            t_ = singles.tile(list(shape), dt, tag=name, name=name)
            if len(shape) == 3:
                # dram [c, 128, f] -> sbuf [128, c, f]
                nc.sync.dma_start(out=t_, in_=src.rearrange("c p f -> p c f"))
            else:
                nc.sync.dma_start(out=t_, in_=src)
            return t_

        wh1_sb = load_single(wh1, [128, 4, 1536], "wh1")
        wh2_sb = load_single(wh2, [128, 4, 1536], "wh2")
        wi1_sb = load_single(wi1, [128, 2, 1536], "wi1")
        wi2_sb = load_single(wi2, [128, 4, 1536], "wi2")
        w1t_sb = load_single(w1t, [128, 4, 128], "w1t")
        bias1_sb = load_single(bias1, [1, 1536], "bias1")
        bias2_sb = load_single(bias2, [1, 1536], "bias2")
        bh1n_sb = load_single(bh1n, [1, 512], "bh1n")
        bh2n_sb = load_single(bh2n, [1, 512], "bh2n")
        b1r_sb = load_single(b1r, [1, 128], "b1r")
        w2_sb = load_single(w2rep, [IB, 128], "w2rep", fp32)
        b2_sb = load_single(b2rep, [IB, 1], "b2rep", fp32)
        ident_sb = load_single(ident, [IB, IB], "ident")
        ident3_sb = load_single(ident3, [72, IB], "ident3")
        ones1_sb = load_single(ones1, [1, 128], "ones1")

        h_sb = [singles.tile([IB, 512], fp16, tag=f"h{k}", name=f"h{k}")
                for k in range(2)]
        # ring buffer of transposed states: slot t%CH holds h(t).T
        CH = min(128, t_steps)
        ht_acc = singles.tile([128, 4, IB, CH], fp16, tag="ht_acc", name="ht_acc")

        # ===================== phase A / C: batched GI =====================
        def gi_phase(lhs_dram, k_chunks, w_sb, bias_sb, gi_out, unroll):
            with ExitStack() as ctx:
                lp = ctx.enter_context(tc.tile_pool(name="gilhs", bufs=3))
                pp = ctx.enter_context(tc.tile_pool(name="gips", bufs=2, space="PSUM"))
                op_ = ctx.enter_context(tc.tile_pool(name="giout", bufs=3))
                with tc.For_i(0, n_mtiles // unroll, 1,
                              hint_engines=(EPE.PE, EPE.SP)) as mb:
                    for u in range(unroll):
                        m = mb * unroll + u
                        lhs = []
                        for k in range(k_chunks):
                            lt = lp.tile([128, 128], fp16, tag=f"lhs{k}",
                                         name=f"lhs{k}")
                            nc.sync.dma_start(out=lt, in_=lhs_dram[k, :, ts(m, 128)])
                            lhs.append(lt)
                        ps = pp.tile([128, 3, 512], fp32, tag="ps")
                        for g in range(3):
                            for k in range(k_chunks):
                                nc.tensor.matmul(ps[:, g, :], lhs[k],
                                                 w_sb[:, k, g * 512:(g + 1) * 512],
                                                 start=(k == 0), stop=False)
                            nc.tensor.matmul(ps[:, g, :], ones1_sb[0:1, :],
                                             bias_sb[0:1, g * 512:(g + 1) * 512],
                                             start=False, stop=True)
                        ot = op_.tile([128, 3, 512], fp16, tag="ot")
                        nc.vector.tensor_copy(ot, ps)
                        for g in range(3):
                            nc.gpsimd.dma_start(out=gi_out[g, ts(m, 128), :],
                                                in_=ot[:, g, :])

        gi_phase(xT, 2, wi1_sb, bias1_sb, gi1, unroll=min(8, n_mtiles))

        # ===================== phase B / D: recurrence =====================
        def recurrence(layer, gi_d, wh_sb, bhn_sb, m1_d, m2_d, hp, htp, h1t_out):
            gi_v = gi_d.rearrange("g (i nb j) f -> g i nb j f", i=IB, j=BLK)
            if h1t_out is not None:
                h1t_v = h1t_out.rearrange("c p (i nc tt) -> c p i nc tt", i=IB, tt=CH)
            blocks_per_chunk = CH // BLK
            n_chunks = t_steps // CH
            with ExitStack() as ctx:
                gp = ctx.enter_context(tc.tile_pool(name="giblk", bufs=3))
                mp = ctx.enter_context(tc.tile_pool(name="mblk", bufs=3))
                pp = ctx.enter_context(tc.tile_pool(name="rpsum", bufs=2, space="PSUM"))
                tp = ctx.enter_context(tc.tile_pool(name="rtmp", bufs=2))
                pt = ctx.enter_context(tc.tile_pool(name="rpsT", bufs=2, space="PSUM"))
                with tc.For_i(0, n_chunks, 1,
                              hint_engines=(EPE.PE, EPE.DVE, EPE.Activation,
                                            EPE.Pool, EPE.SP)) as ic:
                  for jj in range(blocks_per_chunk):
                    ib = ic * blocks_per_chunk + jj
                    gi_blk = gp.tile([72, BLK, 512], fp16, tag="gi", name="gi")
                    for g in range(3):
                        nc.sync.dma_start(
                            out=gi_blk[32 * g:32 * g + IB, :, :],
                            in_=gi_v[g, :, ds(ib, 1), :, :])
                    if layer == 1:
                        mk = mp.tile([IB, BLK], fp32, tag="mk", name="mk")
                        nc.sync.dma_start(out=mk, in_=m1_d[ds(ib, 1), :, :])
                    else:
                        mk = mp.tile([IB, 2, BLK], fp32, tag="mk", name="mk")
                        nc.sync.dma_start(out=mk, in_=m2_d[ds(ib, 1), :, :, :])
                    for j in range(BLK):
                        tslot = jj * BLK + j
                        h_cur, h_nxt = h_sb[(hp + j) % 2], h_sb[(hp + j + 1) % 2]
                        ht_cur = ht_acc[:, :, :, (tslot - 1) % CH]
                        # gh matmul, gates col-tiled to psum partitions {0,32,64};
                        # gi for r,z folded into PSUM via identity matmuls
                        ps = pp.tile([96, 512], fp32, tag="ps")
                        for g in range(3):
                            for k in range(4):
                                nc.tensor.matmul(
                                    ps[32 * g:32 * g + IB, :], ht_cur[:, k, :],
                                    wh_sb[:, k, g * 512:(g + 1) * 512],
                                    start=(k == 0), stop=False)
                            if g != 2:
                                nc.tensor.matmul(
                                    ps[32 * g:32 * g + IB, :],
                                    ident3_sb[32 * g:32 * g + IB, :],
                                    gi_blk[32 * g:32 * g + IB, j, :],
                                    start=False, stop=True)
                        nc.tensor.matmul(ps[64:64 + IB, :], ones1_sb[0:1, 0:IB],
                                         bhn_sb[0:1, :], start=False, stop=True)
                        # gates: sigmoids read summed PSUM directly
                        rbuf = tp.tile([72, 512], fp16, tag="rb")
                        zbuf = tp.tile([IB, 512], fp16, tag="zb")
                        nc.scalar.activation(rbuf[64:64 + IB, :], ps[0:IB, :],
                                             AF.Sigmoid)
                        nc.scalar.activation(zbuf, ps[32:32 + IB, :], AF.Sigmoid)
                        hnb = tp.tile([72, 512], fp16, tag="hnb")
                        nc.scalar.copy(hnb[64:64 + IB, :], ps[64:64 + IB, :])
                        t1 = tp.tile([72, 512], fp16, tag="t1")
                        nc.vector.tensor_mul(t1[64:64 + IB, :], rbuf[64:64 + IB, :],
                                             hnb[64:64 + IB, :])
                        an = tp.tile([72, 512], fp16, tag="an")
                        nc.gpsimd.tensor_tensor(an[64:64 + IB, :], t1[64:64 + IB, :],
                                                gi_blk[64:64 + IB, j, :], OP.add)
                        nbf = tp.tile([IB, 512], fp16, tag="nb")
                        nc.scalar.activation(nbf, an[64:64 + IB, :], AF.Tanh)
                        if layer == 2:
                            # fold freeze into z: z' = z*alive + dead
                            zm = tp.tile([IB, 512], fp16, tag="zm")
                            nc.vector.tensor_scalar(zm, zbuf, mk[:, 0, j:j + 1],
                                                    mk[:, 1, j:j + 1],
                                                    op0=OP.mult, op1=OP.add)
                            zbuf = zm
                        d = tp.tile([IB, 512], fp16, tag="d")
                        nc.gpsimd.tensor_tensor(d, h_cur, nbf, OP.subtract)
                        zd = tp.tile([IB, 512], fp16, tag="zd")
                        nc.vector.tensor_mul(zd, zbuf, d)
                        if layer == 1:
                            h1n = tp.tile([IB, 512], fp16, tag="h1n")
                            nc.vector.tensor_add(h1n, nbf, zd)
                            d2 = tp.tile([IB, 512], fp16, tag="d2")
                            nc.gpsimd.tensor_tensor(d2, h1n, h_cur, OP.subtract)
                            nc.vector.scalar_tensor_tensor(
                                h_nxt, d2, mk[:, j:j + 1], h_cur,
                                op0=OP.mult, op1=OP.add)
                        else:
                            nc.vector.tensor_add(h_nxt, nbf, zd)
                        # transpose bridge: h_nxt [IB,512] -> ht ring slot
                        pst = pt.tile([128, 4, IB], fp16, tag="pst", name="pst")
                        for c in range(4):
                            nc.tensor.transpose(pst[:, c, :],
                                                h_nxt[:, 128 * c:128 * (c + 1)],
                                                ident_sb)
                        nc.vector.tensor_copy(ht_acc[:, :, :, tslot], pst)
                  if h1t_out is not None:
                      # flush the chunk's transposed states (contiguous runs)
                      for c in range(4):
                          nc.gpsimd.dma_start(
                              out=h1t_v[c, :, :, ds(ic, 1), :],
                              in_=ht_acc[:, c, :, :])

        # init states
        nc.sync.dma_start(out=h_sb[0], in_=h10)
        nc.sync.dma_start(out=ht_acc[:, :, :, CH - 1], in_=h10t)
        recurrence(1, gi1, wh1_sb, bh1n_sb, mask1, None, 0, 0, h1t_d)

        if dbg:
            nc.sync.dma_start(out=y_gi1, in_=gi1)
            nc.sync.dma_start(out=y_h1, in_=h_sb[t_steps % 2])
            nc.sync.dma_start(out=y_h1t, in_=h1t_d)

        gi_phase(h1t_d, 4, wi2_sb, bias2_sb, gi2, unroll=min(4, n_mtiles))

        nc.sync.dma_start(out=h_sb[0], in_=h20)
        nc.sync.dma_start(out=ht_acc[:, :, :, CH - 1], in_=h20t)
        recurrence(2, gi2, wh2_sb, bh2n_sb, None, mask2, 0, 0, None)
        if dbg:
            nc.sync.dma_start(out=y_h2, in_=h_sb[t_steps % 2])

        # ===================== phase E: head =====================
        with ExitStack() as ctx:
            ep = ctx.enter_context(tc.tile_pool(name="ep", bufs=1))
            epp = ctx.enter_context(tc.tile_pool(name="epp", bufs=1, space="PSUM"))
            ps1 = epp.tile([IB, 128], fp32)
            hf = h_sb[t_steps % 2]
            htf = ht_acc[:, :, :, (t_steps - 1) % CH]
            for k in range(4):
                nc.tensor.matmul(ps1, htf[:, k, :], w1t_sb[:, k, :],
                                 start=(k == 0), stop=False)
            nc.tensor.matmul(ps1, ones1_sb[0:1, 0:IB], b1r_sb[0:1, :],
                             start=False, stop=True)
            o1 = ep.tile([IB, 128], fp32)
            nc.vector.tensor_copy(o1, ps1)
            t2 = ep.tile([IB, 128], fp32)
            nc.vector.tensor_mul(t2, o1, w2_sb)
            red = ep.tile([IB, 1], fp32)
            nc.vector.tensor_reduce(red, t2, axis=mybir.AxisListType.X, op=OP.add)
            out_t = ep.tile([IB, 1], fp32)
            nc.vector.tensor_scalar(out_t, red, b2_sb[:, 0:1], None, op0=OP.add)
            nc.sync.dma_start(out=y, in_=out_t)

        _ = hf  # silence linter; hf used above

    nc.compile()
    return nc


def _prep_core(c, x, lengths, state0, state1, Wi1, Wh1, bi1, bh1,
               Wi2, Wh2, bi2, bh2, W1, b1, W2, b2, t_steps=T):
    f16 = np.float16
    nb = t_steps // BLK
    xs = np.asarray(x)[c * IB:(c + 1) * IB, :t_steps]        # [IB, T, I]
    ls = np.asarray(lengths)[c * IB:(c + 1) * IB]            # [IB]
    rows = IB * t_steps
    xT = np.ascontiguousarray(
        xs.reshape(rows, I).T.astype(f16)).reshape(2, 128, rows)

    def kstack(WT, kc):       # [K,3H] -> [kc,128,1536]
        return np.ascontiguousarray(WT.astype(f16)).reshape(kc, 128, 1536)

    Wi1T = np.asarray(Wi1).T.astype(np.float32)              # [256, 1536]
    Wh1T = np.asarray(Wh1).T.astype(np.float32)              # [512, 1536]
    Wi2T = np.asarray(Wi2).T.astype(np.float32)
    Wh2T = np.asarray(Wh2).T.astype(np.float32)
    bi1_, bh1_ = np.asarray(bi1, np.float32), np.asarray(bh1, np.float32)
    bi2_, bh2_ = np.asarray(bi2, np.float32), np.asarray(bh2, np.float32)
    bias1 = bi1_.copy(); bias1[:1024] += bh1_[:1024]
    bias2 = bi2_.copy(); bias2[:1024] += bh2_[:1024]

    tgrid = np.arange(t_steps)
    alive = (tgrid[None, :] < ls[:, None]).astype(np.float32)       # [IB, T]
    m1 = np.ascontiguousarray(alive.reshape(IB, nb, BLK).transpose(1, 0, 2))
    m2 = np.stack([alive, 1.0 - alive], axis=1).astype(np.float32)  # [IB,2,T]
    m2 = np.ascontiguousarray(
        m2.reshape(IB, 2, nb, BLK).transpose(2, 0, 1, 3))

    s0 = np.broadcast_to(np.asarray(state0, np.float32), (IB, H)).astype(f16)
    s1 = np.broadcast_to(np.asarray(state1, np.float32), (IB, H)).astype(f16)
    s0t = np.ascontiguousarray(s0.T.reshape(4, 128, IB).transpose(1, 0, 2))
    s1t = np.ascontiguousarray(s1.T.reshape(4, 128, IB).transpose(1, 0, 2))

    W1T = np.asarray(W1).T.astype(f16)                       # [512, 128]
    return {
        "xT": xT,
        "wi1": kstack(Wi1T, 2), "bias1": bias1.astype(f16)[None, :],
        "bh1n": bh1_[1024:].astype(f16)[None, :],
        "wh1": kstack(Wh1T, 4),
        "wi2": kstack(Wi2T, 4), "bias2": bias2.astype(f16)[None, :],
        "bh2n": bh2_[1024:].astype(f16)[None, :],
        "wh2": kstack(Wh2T, 4),
        "w1t": np.ascontiguousarray(W1T).reshape(4, 128, 128),
        "b1r": np.asarray(b1, f16)[None, :],
        "w2rep": np.broadcast_to(np.asarray(W2, np.float32)[0], (IB, 128)).copy(),
        "b2rep": np.full((IB, 1), float(np.asarray(b2).reshape(-1)[0]), np.float32),
        "h10": s0, "h20": s1, "h10t": s0t, "h20t": s1t,
        "ident": np.eye(IB, dtype=f16),
        "ident3": np.concatenate([
            np.concatenate([np.eye(IB), np.zeros((24, IB))], axis=0)
            for _ in range(3)], axis=0)[:72].astype(f16),
        "ones1": np.ones((1, 128), f16),
        "mask1": m1, "mask2": m2,
    }


def kernel(**inputs):
    global _compiled
    from concourse.bass_utils import run_bass_kernel_spmd
    if _compiled is None:
        _compiled = _build()
    in_maps = [_prep_core(c, **inputs) for c in range(NCORES)]
    res = run_bass_kernel_spmd(_compiled, in_maps, core_ids=list(range(NCORES)))
    out = np.concatenate([res.results[c]["y"] for c in range(NCORES)], axis=0)
    return out.astype(np.float32)


# revision 28
# speedup vs baseline: 1.2302x; 1.2108x over previous
"""Bass/Trainium2 kernel for DoubleGRUModel (ragged double GRU + linear head).

Contract: kernel(**inputs) takes FULL unsharded inputs (as numpy-compatible
arrays, keyed as in setup_inputs()) and returns the FULL [64, 1] float32
output. Internally shards the batch across 8 NeuronCores (8 items/core),
runs an SPMD Bass kernel, and gathers.

Per-core pipeline (all phases on-device, no cross-core traffic):
  A: GI1[t,i,:] = x @ Wi1.T + (bi1+bh1 for r,z; bi1 for n)   (batched matmul)
  B: layer-1 time recurrence; stores frozen h1.T per step for phase C
  C: GI2 = h1n @ Wi2.T + biases                               (batched matmul)
  D: layer-2 time recurrence (freeze folded into z-gate)
  E: head: (h2 @ W1.T + b1) @ W2.T + b2
"""

import numpy as np

B, T, I, H = 64, 2048, 256, 512
NCORES = 8
IB = B // NCORES          # items per core = 8
BLK = 8                   # time steps per For_i body
NB = T // BLK             # 256 blocks

_compiled = None


def _build(t_steps=T, dbg=False):
    import concourse.bacc as bacc
    import concourse.bass as bass
    import concourse.mybir as mybir
    import concourse.tile as tile
    from contextlib import ExitStack

    fp16 = mybir.dt.float16
    fp32 = mybir.dt.float32
    AF = mybir.ActivationFunctionType
    OP = mybir.AluOpType
    ds = bass.ds
    ts = bass.ts

    nb = t_steps // BLK
    rows = IB * t_steps          # (i, t)-major flattened rows
    n_mtiles = rows // 128

    nc = bacc.Bacc("TRN2", target_bir_lowering=False, debug=False,
                   num_devices=NCORES)

    def din(name, shape, dt=fp16):
        return nc.dram_tensor(name, list(shape), dt, kind="ExternalInput").ap()

    # ---- external inputs (host-prepped layouts) ----
    xT = din("xT", [2, 128, rows])                   # x.T k-chunks
    wi1 = din("wi1", [2, 128, 1536])                 # Wi1.T k-chunks (gate cols r,z,n)
    bias1 = din("bias1", [1, 1536])                  # bi1 + (bh1 for r,z)
    bh1n = din("bh1n", [1, 512])                     # bh1 n-slice
    wh1 = din("wh1", [4, 128, 1536])                 # Wh1.T k-chunks
    wi2 = din("wi2", [4, 128, 1536])
    bias2 = din("bias2", [1, 1536])
    bh2n = din("bh2n", [1, 512])
    wh2 = din("wh2", [4, 128, 1536])
    w1t = din("w1t", [4, 128, 128])                  # W1.T
    b1r = din("b1r", [1, 128])
    w2rep = din("w2rep", [IB, 128], fp32)            # W2 row replicated
    b2rep = din("b2rep", [IB, 1], fp32)
    h10 = din("h10", [IB, 512])                      # initial states (bcast)
    h20 = din("h20", [IB, 512])
    h10t = din("h10t", [128, 4, IB])                 # same, transposed
    h20t = din("h20t", [128, 4, IB])
    ident = din("ident", [IB, IB])                   # identity matrix
    ident3 = din("ident3", [72, IB])                 # identity at partitions {0,32,64}
    ones1 = din("ones1", [1, 128])                   # ones rows
    mask1 = din("mask1", [nb, IB, BLK], fp32)              # alive(t)=1 else 0
    mask2 = din("mask2", [nb, IB, 2, BLK], fp32)           # [alive, dead]
    y = nc.dram_tensor("y", [IB, 1], fp32, kind="ExternalOutput").ap()
    if dbg:
        rows_ = IB * t_steps
        y_gi1 = nc.dram_tensor("y_gi1", [3, rows_, 512], fp16,
                               kind="ExternalOutput").ap()
        y_h1 = nc.dram_tensor("y_h1", [IB, 512], fp16,
                              kind="ExternalOutput").ap()
        y_h1t = nc.dram_tensor("y_h1t", [4, 128, rows_], fp16,
                               kind="ExternalOutput").ap()
        y_h2 = nc.dram_tensor("y_h2", [IB, 512], fp16,
                              kind="ExternalOutput").ap()

    # ---- internal DRAM ----
    gi1 = nc.dram_tensor("gi1", [3, rows, 512], fp16).ap()
    gi2 = nc.dram_tensor("gi2", [3, rows, 512], fp16).ap()
    h1t_d = nc.dram_tensor("h1t_d", [4, 128, rows], fp16).ap()

    EPE = mybir.EngineType

    with tile.TileContext(nc) as tc, ExitStack() as top:
        singles = top.enter_context(tc.tile_pool(name="singles", bufs=1))

        # ---- persistent SBUF: weights, constants, states ----
        def load_single(src, shape, name, dt=fp16):
            t_ = singles.tile(list(shape), dt, tag=name, name=name)
            if len(shape) == 3:
                # dram [c, 128, f] -> sbuf [128, c, f]
                nc.sync.dma_start(out=t_, in_=src.rearrange("c p f -> p c f"))
            else:
                nc.sync.dma_start(out=t_, in_=src)
            return t_

        wh1_sb = load_single(wh1, [128, 4, 1536], "wh1")
        wh2_sb = load_single(wh2, [128, 4, 1536], "wh2")
        wi1_sb = load_single(wi1, [128, 2, 1536], "wi1")
        wi2_sb = load_single(wi2, [128, 4, 1536], "wi2")
        w1t_sb = load_single(w1t, [128, 4, 128], "w1t")
        bias1_sb = load_single(bias1, [1, 1536], "bias1")
        bias2_sb = load_single(bias2, [1, 1536], "bias2")
        bh1n_sb = load_single(bh1n, [1, 512], "bh1n")
        bh2n_sb = load_single(bh2n, [1, 512], "bh2n")
        b1r_sb = load_single(b1r, [1, 128], "b1r")
        w2_sb = load_single(w2rep, [IB, 128], "w2rep", fp32)
        b2_sb = load_single(b2rep, [IB, 1], "b2rep", fp32)
        ident_sb = load_single(ident, [IB, IB], "ident")
        ident3_sb = load_single(ident3, [72, IB], "ident3")
        ones1_sb = load_single(ones1, [1, 128], "ones1")

        h_sb = [singles.tile([IB, 512], fp16, tag=f"h{k}", name=f"h{k}")
                for k in range(2)]
        # ring buffer of transposed states: slot t%CH holds h(t).T
        CH = min(128, t_steps)
        ht_acc = singles.tile([128, 4, IB, CH], fp16, tag="ht_acc", name="ht_acc")

        # ===================== phase A / C: batched GI =====================
        def gi_phase(lhs_dram, k_chunks, w_sb, bias_sb, gi_out, unroll):
            with ExitStack() as ctx:
                lp = ctx.enter_context(tc.tile_pool(name="gilhs", bufs=3))
                pp = ctx.enter_context(tc.tile_pool(name="gips", bufs=2, space="PSUM"))
                op_ = ctx.enter_context(tc.tile_pool(name="giout", bufs=3))
                with tc.For_i(0, n_mtiles // unroll, 1,
                              hint_engines=(EPE.PE, EPE.SP)) as mb:
                    for u in range(unroll):
                        m = mb * unroll + u
                        lhs = []
                        for k in range(k_chunks):
                            lt = lp.tile([128, 128], fp16, tag=f"lhs{k}",
                                         name=f"lhs{k}")
                            nc.sync.dma_start(out=lt, in_=lhs_dram[k, :, ts(m, 128)])
                            lhs.append(lt)
                        ps = pp.tile([128, 3, 512], fp32, tag="ps")
                        for g in range(3):
                            for k in range(k_chunks):
                                nc.tensor.matmul(ps[:, g, :], lhs[k],
                                                 w_sb[:, k, g * 512:(g + 1) * 512],
                                                 start=(k == 0), stop=False)
                            nc.tensor.matmul(ps[:, g, :], ones1_sb[0:1, :],
                                             bias_sb[0:1, g * 512:(g + 1) * 512],
                                             start=False, stop=True)
                        ot = op_.tile([128, 3, 512], fp16, tag="ot")
                        nc.vector.tensor_copy(ot, ps)
                        for g in range(3):
                            nc.gpsimd.dma_start(out=gi_out[g, ts(m, 128), :],
                                                in_=ot[:, g, :])

        gi_phase(xT, 2, wi1_sb, bias1_sb, gi1, unroll=min(8, n_mtiles))

        # ===================== phase B / D: recurrence =====================
        def recurrence(layer, gi_d, wh_sb, bhn_sb, m1_d, m2_d, hp, htp, h1t_out):
            gi_v = gi_d.rearrange("g (i nb j) f -> g i nb j f", i=IB, j=BLK)
            if h1t_out is not None:
                h1t_v = h1t_out.rearrange("c p (i nc tt) -> c p i nc tt", i=IB, tt=CH)
            blocks_per_chunk = CH // BLK
            n_chunks = t_steps // CH
            with ExitStack() as ctx:
                gp = ctx.enter_context(tc.tile_pool(name="giblk", bufs=3))
                mp = ctx.enter_context(tc.tile_pool(name="mblk", bufs=3))
                pp = ctx.enter_context(tc.tile_pool(name="rpsum", bufs=2, space="PSUM"))
                tp = ctx.enter_context(tc.tile_pool(name="rtmp", bufs=2))
                pt = ctx.enter_context(tc.tile_pool(name="rpsT", bufs=2, space="PSUM"))
                with tc.For_i(0, n_chunks, 1,
                              hint_engines=(EPE.PE, EPE.DVE, EPE.Activation,
                                            EPE.Pool, EPE.SP)) as ic:
                  for jj in range(blocks_per_chunk):
                    ib = ic * blocks_per_chunk + jj
                    gi_blk = gp.tile([72, BLK, 512], fp16, tag="gi", name="gi")
                    for g in range(3):
                        nc.sync.dma_start(
                            out=gi_blk[32 * g:32 * g + IB, :, :],
                            in_=gi_v[g, :, ds(ib, 1), :, :])
                    if layer == 1:
                        mk = mp.tile([IB, BLK], fp32, tag="mk", name="mk")
                        nc.sync.dma_start(out=mk, in_=m1_d[ds(ib, 1), :, :])
                    else:
                        mk = mp.tile([IB, 2, BLK], fp32, tag="mk", name="mk")
                        nc.sync.dma_start(out=mk, in_=m2_d[ds(ib, 1), :, :, :])
                    for j in range(BLK):
                        tslot = jj * BLK + j
                        h_cur, h_nxt = h_sb[(hp + j) % 2], h_sb[(hp + j + 1) % 2]
                        ht_cur = ht_acc[:, :, :, (tslot - 1) % CH]
                        # gh matmul, gates col-tiled to psum partitions {0,32,64};
                        # gi for r,z folded into PSUM via identity matmuls
                        ps = pp.tile([96, 512], fp32, tag="ps")
                        for g in range(3):
                            for k in range(4):
                                nc.tensor.matmul(
                                    ps[32 * g:32 * g + IB, :], ht_cur[:, k, :],
                                    wh_sb[:, k, g * 512:(g + 1) * 512],
                                    start=(k == 0), stop=False)
                            if g != 2:
                                nc.tensor.matmul(
                                    ps[32 * g:32 * g + IB, :],
                                    ident3_sb[32 * g:32 * g + IB, :],
                                    gi_blk[32 * g:32 * g + IB, j, :],
                                    start=False, stop=True)
                        nc.tensor.matmul(ps[64:64 + IB, :], ones1_sb[0:1, 0:IB],
                                         bhn_sb[0:1, :], start=False, stop=True)
                        # gates: sigmoids read summed PSUM directly
                        rbuf = tp.tile([72, 512], fp16, tag="rb")
                        zbuf = tp.tile([IB, 512], fp16, tag="zb")
                        nc.scalar.activation(rbuf[64:64 + IB, :], ps[0:IB, :],
                                             AF.Sigmoid)
                        nc.scalar.activation(zbuf, ps[32:32 + IB, :], AF.Sigmoid)
                        hnb = tp.tile([72, 512], fp16, tag="hnb")
                        nc.scalar.copy(hnb[64:64 + IB, :], ps[64:64 + IB, :])
                        t1 = tp.tile([72, 512], fp16, tag="t1")
                        nc.vector.tensor_mul(t1[64:64 + IB, :], rbuf[64:64 + IB, :],
                                             hnb[64:64 + IB, :])
                        an = tp.tile([72, 512], fp16, tag="an")
                        nc.vector.tensor_tensor(an[64:64 + IB, :], t1[64:64 + IB, :],
                                                gi_blk[64:64 + IB, j, :], OP.add)
                        nbf = tp.tile([IB, 512], fp16, tag="nb")
                        nc.scalar.activation(nbf, an[64:64 + IB, :], AF.Tanh)
                        if layer == 2:
                            # fold freeze into z: z' = z*alive + dead
                            zm = tp.tile([IB, 512], fp16, tag="zm")
                            nc.vector.tensor_scalar(zm, zbuf, mk[:, 0, j:j + 1],
                                                    mk[:, 1, j:j + 1],
                                                    op0=OP.mult, op1=OP.add)
                            zbuf = zm
                        d = tp.tile([IB, 512], fp16, tag="d")
                        nc.vector.tensor_tensor(d, h_cur, nbf, OP.subtract)
                        zd = tp.tile([IB, 512], fp16, tag="zd")
                        nc.vector.tensor_mul(zd, zbuf, d)
                        if layer == 1:
                            # h' = h + alive*(h1n - h); h1n - h = zd - d
                            d2 = tp.tile([IB, 512], fp16, tag="d2")
                            nc.vector.tensor_tensor(d2, zd, d, OP.subtract)
                            nc.vector.scalar_tensor_tensor(
                                h_nxt, d2, mk[:, j:j + 1], h_cur,
                                op0=OP.mult, op1=OP.add)
                        else:
                            nc.vector.tensor_add(h_nxt, nbf, zd)
                        # transpose bridge: h_nxt [IB,512] -> ht ring slot
                        pst = pt.tile([128, 4, IB], fp16, tag="pst", name="pst")
                        for c in range(4):
                            nc.tensor.transpose(pst[:, c, :],
                                                h_nxt[:, 128 * c:128 * (c + 1)],
                                                ident_sb)
                        nc.vector.tensor_copy(ht_acc[:, :, :, tslot], pst)
                  if h1t_out is not None:
                      # flush the chunk's transposed states (contiguous runs)
                      for c in range(4):
                          nc.gpsimd.dma_start(
                              out=h1t_v[c, :, :, ds(ic, 1), :],
                              in_=ht_acc[:, c, :, :])

        # init states
        nc.sync.dma_start(out=h_sb[0], in_=h10)
        nc.sync.dma_start(out=ht_acc[:, :, :, CH - 1], in_=h10t)
        recurrence(1, gi1, wh1_sb, bh1n_sb, mask1, None, 0, 0, h1t_d)

        if dbg:
            nc.sync.dma_start(out=y_gi1, in_=gi1)
            nc.sync.dma_start(out=y_h1, in_=h_sb[t_steps % 2])
            nc.sync.dma_start(out=y_h1t, in_=h1t_d)

        gi_phase(h1t_d, 4, wi2_sb, bias2_sb, gi2, unroll=min(4, n_mtiles))

        nc.sync.dma_start(out=h_sb[0], in_=h20)
        nc.sync.dma_start(out=ht_acc[:, :, :, CH - 1], in_=h20t)
        recurrence(2, gi2, wh2_sb, bh2n_sb, None, mask2, 0, 0, None)
        if dbg:
            nc.sync.dma_start(out=y_h2, in_=h_sb[t_steps % 2])

        # ===================== phase E: head =====================
        with ExitStack() as ctx:
            ep = ctx.enter_context(tc.tile_pool(name="ep", bufs=1))
            epp = ctx.enter_context(tc.tile_pool(name="epp", bufs=1, space="PSUM"))
            ps1 = epp.tile([IB, 128], fp32)
            hf = h_sb[t_steps % 2]
            htf = ht_acc[:, :, :, (t_steps - 1) % CH]
            for k in range(4):
                nc.tensor.matmul(ps1, htf[:, k, :], w1t_sb[:, k, :],
                                 start=(k == 0), stop=False)
            nc.tensor.matmul(ps1, ones1_sb[0:1, 0:IB], b1r_sb[0:1, :],
                             start=False, stop=True)
            o1 = ep.tile([IB, 128], fp32)
            nc.vector.tensor_copy(o1, ps1)
            t2 = ep.tile([IB, 128], fp32)
            nc.vector.tensor_mul(t2, o1, w2_sb)
            red = ep.tile([IB, 1], fp32)
            nc.vector.tensor_reduce(red, t2, axis=mybir.AxisListType.X, op=OP.add)
            out_t = ep.tile([IB, 1], fp32)
            nc.vector.tensor_scalar(out_t, red, b2_sb[:, 0:1], None, op0=OP.add)
            nc.sync.dma_start(out=y, in_=out_t)

        _ = hf  # silence linter; hf used above

    nc.compile()
    return nc


def _prep_core(c, x, lengths, state0, state1, Wi1, Wh1, bi1, bh1,
               Wi2, Wh2, bi2, bh2, W1, b1, W2, b2, t_steps=T):
    f16 = np.float16
    nb = t_steps // BLK
    xs = np.asarray(x)[c * IB:(c + 1) * IB, :t_steps]        # [IB, T, I]
    ls = np.asarray(lengths)[c * IB:(c + 1) * IB]            # [IB]
    rows = IB * t_steps
    xT = np.ascontiguousarray(
        xs.reshape(rows, I).T.astype(f16)).reshape(2, 128, rows)

    def kstack(WT, kc):       # [K,3H] -> [kc,128,1536]
        return np.ascontiguousarray(WT.astype(f16)).reshape(kc, 128, 1536)

    Wi1T = np.asarray(Wi1).T.astype(np.float32)              # [256, 1536]
    Wh1T = np.asarray(Wh1).T.astype(np.float32)              # [512, 1536]
    Wi2T = np.asarray(Wi2).T.astype(np.float32)
    Wh2T = np.asarray(Wh2).T.astype(np.float32)
    bi1_, bh1_ = np.asarray(bi1, np.float32), np.asarray(bh1, np.float32)
    bi2_, bh2_ = np.asarray(bi2, np.float32), np.asarray(bh2, np.float32)
    bias1 = bi1_.copy(); bias1[:1024] += bh1_[:1024]
    bias2 = bi2_.copy(); bias2[:1024] += bh2_[:1024]

    tgrid = np.arange(t_steps)
    alive = (tgrid[None, :] < ls[:, None]).astype(np.float32)       # [IB, T]
    m1 = np.ascontiguousarray(alive.reshape(IB, nb, BLK).transpose(1, 0, 2))
    m2 = np.stack([alive, 1.0 - alive], axis=1).astype(np.float32)  # [IB,2,T]
    m2 = np.ascontiguousarray(
        m2.reshape(IB, 2, nb, BLK).transpose(2, 0, 1, 3))

    s0 = np.broadcast_to(np.asarray(state0, np.float32), (IB, H)).astype(f16)
    s1 = np.broadcast_to(np.asarray(state1, np.float32), (IB, H)).astype(f16)
    s0t = np.ascontiguousarray(s0.T.reshape(4, 128, IB).transpose(1, 0, 2))
    s1t = np.ascontiguousarray(s1.T.reshape(4, 128, IB).transpose(1, 0, 2))

    W1T = np.asarray(W1).T.astype(f16)                       # [512, 128]
    return {
        "xT": xT,
        "wi1": kstack(Wi1T, 2), "bias1": bias1.astype(f16)[None, :],
        "bh1n": bh1_[1024:].astype(f16)[None, :],
        "wh1": kstack(Wh1T, 4),
        "wi2": kstack(Wi2T, 4), "bias2": bias2.astype(f16)[None, :],
        "bh2n": bh2_[1024:].astype(f16)[None, :],
        "wh2": kstack(Wh2T, 4),
        "w1t": np.ascontiguousarray(W1T).reshape(4, 128, 128),
        "b1r": np.asarray(b1, f16)[None, :],
        "w2rep": np.broadcast_to(np.asarray(W2, np.float32)[0], (IB, 128)).copy(),
        "b2rep": np.full((IB, 1), float(np.asarray(b2).reshape(-1)[0]), np.float32),
        "h10": s0, "h20": s1, "h10t": s0t, "h20t": s1t,
        "ident": np.eye(IB, dtype=f16),
        "ident3": np.concatenate([
            np.concatenate([np.eye(IB), np.zeros((24, IB))], axis=0)
            for _ in range(3)], axis=0)[:72].astype(f16),
        "ones1": np.ones((1, 128), f16),
        "mask1": m1, "mask2": m2,
    }


def kernel(**inputs):
    global _compiled
    from concourse.bass_utils import run_bass_kernel_spmd
    if _compiled is None:
        _compiled = _build()
    in_maps = [_prep_core(c, **inputs) for c in range(NCORES)]
    res = run_bass_kernel_spmd(_compiled, in_maps, core_ids=list(range(NCORES)))
    out = np.concatenate([res.results[c]["y"] for c in range(NCORES)], axis=0)
    return out.astype(np.float32)
